# revision 1
# baseline (speedup 1.0000x reference)
"""Trainium2 Bass kernel for nn_MultiHeadAttention (B=4, L=S=2048, D=1024, H=16, causal).

Sharding: 8 cores = 4 batches x 2 head-groups (8 heads each).
Per core: project its batch's q/k/v against its group's weight slices,
causal attention for 8 heads, output-projection against Wo column slice.
Host sums the 2 partial outputs per batch (tensor-parallel reduce).

All matmuls in bf16 with fp32 PSUM accumulation.
Inputs are pre-rounded to bf16 on the host during the shard step (the kernel
computes in bf16 regardless); activations are transposed to [D, tokens] layout
on-chip via DMA-transpose directly from DRAM.
"""

import sys

if "/opt/trn_rl_repo" not in sys.path:
    sys.path.insert(0, "/opt/trn_rl_repo")

import numpy as np
import ml_dtypes

BF16 = ml_dtypes.bfloat16

# Problem constants (hardcoded per harness contract)
B, L, D, H = 4, 2048, 1024, 16
HD = D // H              # 64
NCORES = 8
GROUPS = 2               # head-groups (tensor parallel)
HG = H // GROUPS         # 8 heads per group
DG = HG * HD             # 512 out-dim per group

# Full-size device config
FULL_CFG = dict(T=L, DM=D, DG=DG)


def emit_mha(tc, aps, cfg):
    """Emit the per-core MHA program into TileContext tc.

    aps: dict of bass APs: xq, xk, xv, wq, wk, wv, wo, maskt (inputs), y (output)
    cfg: dict(T, DM, DG)
    """
    import concourse.bass as bass
    from concourse import mybir

    nc = tc.nc
    f32 = mybir.dt.float32
    bf16 = mybir.dt.bfloat16
    Exp = mybir.ActivationFunctionType.Exp

    T, DM, DG_ = cfg["T"], cfg["DM"], cfg["DG"]
    TB = 128                  # s/l block
    LCH = min(512, T)         # l-chunk (moving-dim)
    nDch = DM // 128          # D chunks (contraction)
    nTt = T // TB             # token tiles
    nLch = T // LCH           # l-chunks
    nMask = LCH // TB         # diagonal mask tiles
    nPair = DG_ // 128        # head pairs (2 heads of 64 per pair)
    OCH = min(512, DM)        # Wo output chunk
    nOch = DM // OCH          # output chunks for Wo
    SCALE = 1.0 / np.sqrt(HD)

    import contextlib

    ctx = contextlib.ExitStack()
    with ctx:
        dram = ctx.enter_context(tc.tile_pool(name="dram", bufs=1, space="DRAM"))
        wpool = ctx.enter_context(tc.tile_pool(name="wts", bufs=1))
        xt_pool = ctx.enter_context(tc.tile_pool(name="xt", bufs=2 * nDch))
        qkv_pool = ctx.enter_context(tc.tile_pool(name="qkv", bufs=1))
        pt_pool = ctx.enter_context(tc.tile_pool(name="pt", bufs=4))
        ctxt_pool = ctx.enter_context(tc.tile_pool(name="ctxt", bufs=1))
        small = ctx.enter_context(tc.tile_pool(name="small", bufs=2))
        outsb_pool = ctx.enter_context(tc.tile_pool(name="outsb", bufs=2))
        # PSUM: st 2x2 banks + ctx 2 + sums 1 + proj 1 = 8 banks
        st_ps = ctx.enter_context(tc.tile_pool(name="st_ps", bufs=2, space="PSUM"))
        ctx_ps_pool = ctx.enter_context(tc.tile_pool(name="ctx_ps", bufs=2, space="PSUM"))
        sums_ps_pool = ctx.enter_context(tc.tile_pool(name="sums_ps", bufs=1, space="PSUM"))
        proj_ps = ctx.enter_context(tc.tile_pool(name="proj_ps", bufs=1, space="PSUM"))

        # ---- constants ----
        ones = wpool.tile([128, 1], bf16, tag="ones")
        nc.vector.memset(ones[:], 1.0)

        # ---- inputs arrive bf16 from the host shard step; transpose them
        # straight out of DRAM (DMA-transpose, no staging, no deps).
        def wtrans(dst_tag, src):
            tiles = []
            for c in range(nDch):
                t = wpool.tile([128, DG_], bf16, tag=f"{dst_tag}{c}")
                nc.sync.dma_start(out=t[:], in_=src[:, c * 128:(c + 1) * 128], transpose=True)
                tiles.append(t)
            return tiles

        def xtrans(src):
            tiles = []
            for c in range(nDch):
                t = xt_pool.tile([128, T], bf16, tag="xt")
                nc.sync.dma_start(out=t[:], in_=src[:, c * 128:(c + 1) * 128], transpose=True)
                tiles.append(t)
            return tiles

        # sync-queue order = consumption order
        wvT = wtrans("wvT", aps["wv"])
        vT = xtrans(aps["xv"])
        masks = []
        for r in range(nMask):
            mt = wpool.tile([TB, 2 * LCH], bf16, tag=f"mask{r}")
            nc.sync.dma_start(out=mt[:], in_=aps["maskt"][r])
            masks.append(mt)
        wqT = wtrans("wqT", aps["wq"])
        qT = xtrans(aps["xq"])
        wkT = wtrans("wkT", aps["wk"])
        kT = xtrans(aps["xk"])
        woT = []
        for c in range(DG_ // 128):
            t = wpool.tile([128, DM], bf16, tag=f"woT{c}")
            nc.sync.dma_start(out=t[:], in_=aps["wo"][:, c * 128:(c + 1) * 128], transpose=True)
            woT.append(t)

        # ---- V projection: V[st] [128, DG] natural (s on partitions) ----
        V = []
        for st in range(nTt):
            ps = proj_ps.tile([128, min(512, DG_)], f32, tag="proj")
            # DG_ <= 512 assumed (one N chunk)
            for c in range(nDch):
                nc.tensor.matmul(ps[:], lhsT=vT[c][:, st * TB:(st + 1) * TB],
                                 rhs=wvT[c][:], start=(c == 0), stop=(c == nDch - 1))
            vt = qkv_pool.tile([128, DG_], bf16, tag=f"V{st}")
            nc.vector.tensor_copy(vt[:], ps[:])
            V.append(vt)

        # ---- QT projection: QT[m][n] [128, LCH] fine tiles so attention
        # chunks unblock as soon as their slice is projected ----
        QT = []
        for m in range(nPair):
            qts = []
            for n in range(nLch):
                ps = proj_ps.tile([128, LCH], f32, tag="proj")
                for c in range(nDch):
                    nc.tensor.matmul(ps[:], lhsT=wqT[c][:, m * 128:(m + 1) * 128],
                                     rhs=qT[c][:, n * LCH:(n + 1) * LCH],
                                     start=(c == 0), stop=(c == nDch - 1))
                qtn = qkv_pool.tile([128, LCH], bf16, tag=f"QT{m}_{n}", name=f"QT{m}_{n}")
                nc.vector.tensor_copy(qtn[:], ps[:])
                qts.append(qtn)
            QT.append(qts)

        ctxT = [[None] * nLch for _ in range(nPair)]
        cpc_pool = ctx.enter_context(tc.tile_pool(name="cpc", bufs=3))
        kt_pool = ctx.enter_context(tc.tile_pool(name="ktp", bufs=2))
        tiny = ctx.enter_context(tc.tile_pool(name="tiny", bufs=1))
        for p in range(nPair):
            # KT[p] projection (fine tiles; slots recycle across pairs)
            ktn = []
            for n in range(nLch):
                ps = proj_ps.tile([128, LCH], f32, tag="proj")
                for c in range(nDch):
                    nc.tensor.matmul(ps[:], lhsT=wkT[c][:, p * 128:(p + 1) * 128],
                                     rhs=kT[c][:, n * LCH:(n + 1) * LCH],
                                     start=(c == 0), stop=(c == nDch - 1))
                kt_t = kt_pool.tile([128, LCH], bf16, tag=f"KT{n}", name=f"KT{n}_{p}")
                nc.vector.tensor_copy(kt_t[:], ps[:])
                ktn.append(kt_t)
            qts = QT[p]

            # attention for this pair of heads
            pair_sums = small.tile([2 * nLch, LCH], f32, tag="psums")
            cpcs = []
            for i in range(nLch):
                nsb = (i + 1) * (LCH // TB)
                cps = ctx_ps_pool.tile([128, LCH], f32, tag="ctx")
                sps = sums_ps_pool.tile([33, LCH], f32, tag="sums")
                for j in range(nsb):
                    ktj = ktn[j // (LCH // TB)]
                    koff = (j % (LCH // TB)) * TB
                    sp = st_ps.tile([128, 2 * LCH], f32, tag="st")
                    nc.tensor.matmul(sp[:, 0:LCH],
                                     lhsT=ktj[0:64, koff:koff + TB],
                                     rhs=qts[i][0:64, :],
                                     start=True, stop=True)
                    nc.tensor.matmul(sp[:, LCH:2 * LCH],
                                     lhsT=ktj[64:128, koff:koff + TB],
                                     rhs=qts[i][64:128, :],
                                     start=True, stop=True)
                    pt = pt_pool.tile([128, 2 * LCH], bf16, tag="pt")
                    nc.scalar.activation(pt[:], sp[:], Exp, scale=float(SCALE))
                    r = j - (LCH // TB) * i
                    if r >= 0:
                        nc.vector.tensor_mul(pt[:], pt[:], masks[r][:])
                    st = (j == 0)
                    en = (j == nsb - 1)
                    nc.tensor.matmul(cps[0:64, :], lhsT=V[j][:, p * 128:p * 128 + 64],
                                     rhs=pt[:, 0:LCH], start=st, stop=en,
                                     skip_group_check=True)
                    nc.tensor.matmul(cps[64:128, :], lhsT=V[j][:, p * 128 + 64:p * 128 + 128],
                                     rhs=pt[:, LCH:2 * LCH], start=st, stop=en,
                                     skip_group_check=True)
                    nc.tensor.matmul(sps[0:1, :], lhsT=ones[:], rhs=pt[:, 0:LCH],
                                     start=st, stop=en, skip_group_check=True)
                    nc.tensor.matmul(sps[32:33, :], lhsT=ones[:], rhs=pt[:, LCH:2 * LCH],
                                     start=st, stop=en, skip_group_check=True)
                # Drain both PSUM accumulators to SBUF immediately (frees the
                # banks so the next l-chunk's matmuls never stall and the PE
                # never idles into a HAM re-throttle). Normalization happens
                # off the critical path, batched per pair.
                cpc = cpc_pool.tile([128, LCH], f32, tag="cpc")
                nc.vector.tensor_copy(cpc[:], cps[:])
                cpcs.append(cpc)
                srow = small.tile([33, LCH], f32, tag="srow")
                nc.vector.tensor_copy(srow[0:1, :], sps[0:1, :])
                nc.vector.tensor_copy(srow[32:33, :], sps[32:33, :])
                nc.gpsimd.dma_start(out=pair_sums[2 * i:2 * i + 1, :], in_=srow[0:1, :])
                nc.gpsimd.dma_start(out=pair_sums[2 * i + 1:2 * i + 2, :], in_=srow[32:33, :])

            # one batched exact reciprocal for the whole pair (DVE serial cost
            # is per-lane free-size, so [8, LCH] costs the same as [1, LCH])
            pair_rec = small.tile([2 * nLch, LCH], f32, tag="prec")
            nc.vector.reciprocal(pair_rec[:], pair_sums[:])
            for i in range(nLch):
                # gather the two recip rows to partition 0 (partition_broadcast
                # needs src base 0 on HW); tiny DMAs stay off the sync queue.
                rec01 = tiny.tile([1, 2 * LCH], f32, tag="rec01")
                nc.gpsimd.dma_start(out=rec01[0:1, 0:LCH], in_=pair_rec[2 * i:2 * i + 1, :])
                nc.gpsimd.dma_start(out=rec01[0:1, LCH:2 * LCH], in_=pair_rec[2 * i + 1:2 * i + 2, :])
                rb = tiny.tile([128, 2 * LCH], f32, tag="rb")
                nc.gpsimd.partition_broadcast(rb[:], rec01[0:1, :])
                ct = ctxt_pool.tile([128, LCH], bf16, tag=f"ctxT{p}_{i}")
                nc.vector.tensor_mul(ct[0:64, :], cpcs[i][0:64, :], rb[0:64, 0:LCH])
                nc.vector.tensor_mul(ct[64:128, :], cpcs[i][64:128, :], rb[64:128, LCH:2 * LCH])
                ctxT[p][i] = ct

        # ---- Wo: y[lt*128:, :] = ctx @ WoT ----
        for lt in range(nTt):
            osb = outsb_pool.tile([128, DM], f32, tag="osb")
            for oc in range(nOch):
                ps = proj_ps.tile([128, OCH], f32, tag="proj")
                for dc in range(nPair):
                    lhsT = ctxT[dc][lt // nMask][:, (lt % nMask) * TB:(lt % nMask) * TB + TB]
                    nc.tensor.matmul(ps[:], lhsT=lhsT,
                                     rhs=woT[dc][:, oc * OCH:(oc + 1) * OCH],
                                     start=(dc == 0), stop=(dc == nPair - 1))
                nc.vector.tensor_copy(osb[:, oc * OCH:(oc + 1) * OCH], ps[:])
            nc.sync.dma_start(out=aps["y"][lt * TB:(lt + 1) * TB, :], in_=osb[:])


def make_mask_tiles(cfg):
    T, LCH, TB = cfg["T"], min(512, cfg["T"]), 128
    nMask = LCH // TB
    f = np.arange(2 * LCH) % LCH
    p = np.arange(TB)
    tiles = []
    for r in range(nMask):
        m = (f[None, :] >= (TB * r + p)[:, None]).astype(np.float32)
        tiles.append(m)
    return np.stack(tiles).astype(BF16)


def build_nc(cfg):
    """Build and compile the per-core Bass program. Returns (nc, input_names)."""
    import concourse.bacc as bacc
    import concourse.tile as tile
    from concourse import mybir

    T, DM, DG_ = cfg["T"], cfg["DM"], cfg["DG"]
    LCH = min(512, T)
    nMask = LCH // 128

    nc = bacc.Bacc("TRN2", target_bir_lowering=False, debug=False)
    f32 = mybir.dt.float32
    bf16 = mybir.dt.bfloat16
    aps = {}
    for nm, shape, dt in [
        ("xq", [T, DM], bf16), ("xk", [T, DM], bf16), ("xv", [T, DM], bf16),
        ("wq", [DG_, DM], bf16), ("wk", [DG_, DM], bf16), ("wv", [DG_, DM], bf16),
        ("wo", [DM, DG_], bf16),
        ("maskt", [nMask, 128, 2 * LCH], bf16),
    ]:
        aps[nm] = nc.dram_tensor(nm, shape, dt, kind="ExternalInput").ap()
    aps["y"] = nc.dram_tensor("y", [T, DM], f32, kind="ExternalOutput").ap()

    with tile.TileContext(nc) as tc:
        emit_mha(tc, aps, cfg)
    nc.compile()
    return nc


_CACHE = {}


def _get_nc():
    if "nc" not in _CACHE:
        _CACHE["nc"] = build_nc(FULL_CFG)
    return _CACHE["nc"]


def shard_inputs(q, k, v, Wq, Wk, Wv, Wo):
    """Build the per-core input maps (8 cores = 4 batches x 2 groups)."""
    maskt = make_mask_tiles(FULL_CFG)
    in_maps = []
    for core in range(NCORES):
        b, g = divmod(core, GROUPS)
        rows = slice(g * DG, (g + 1) * DG)
        in_maps.append({
            "xq": np.ascontiguousarray(q[b]).astype(BF16),
            "xk": np.ascontiguousarray(k[b]).astype(BF16),
            "xv": np.ascontiguousarray(v[b]).astype(BF16),
            "wq": np.ascontiguousarray(Wq[rows]).astype(BF16),
            "wk": np.ascontiguousarray(Wk[rows]).astype(BF16),
            "wv": np.ascontiguousarray(Wv[rows]).astype(BF16),
            "wo": np.ascontiguousarray(Wo[:, rows]).astype(BF16),
            "maskt": maskt,
        })
    return in_maps


def kernel(q, k, v, mask, Wq, Wk, Wv, Wo):
    from concourse import bass_utils

    q = np.asarray(q, dtype=np.float32)
    k = np.asarray(k, dtype=np.float32)
    v = np.asarray(v, dtype=np.float32)
    Wq = np.asarray(Wq, dtype=np.float32)
    Wk = np.asarray(Wk, dtype=np.float32)
    Wv = np.asarray(Wv, dtype=np.float32)
    Wo = np.asarray(Wo, dtype=np.float32)

    nc = _get_nc()
    in_maps = shard_inputs(q, k, v, Wq, Wk, Wv, Wo)
    res = bass_utils.run_bass_kernel_spmd(nc, in_maps, core_ids=list(range(NCORES)))
    out = np.zeros((B, L, D), dtype=np.float32)
    for core in range(NCORES):
        b = core // GROUPS
        out[b] += res.results[core]["y"]
    return out

